# revision 1
# baseline (speedup 1.0000x reference)
"""ConvGRU Trainium2 kernel (8 NeuronCores, SPMD).

Problem: T=10, N=4, CIN=64, C=128, H=W=64.
  y = BN(conv5x5(x))  over T*N batch  -> GRU scan over T with conv3x3 gates.

Sharding: 8 cores = N(4) x H-halves(2). Core j: n=j//2, half=j%2,
rows [r0,r1) = [0,32) or [32,64). Phase 1 (x2h conv + BN stats) is fully
local per core (input halos come free from host-side slicing); BN stats
need one 8-core AllReduce; the scan exchanges a 2-row h halo between
H-half partners each step (pairwise AllGather).

The program is identical on all cores (SPMD). Half-dependent behavior
(which halo rows are real vs zero at the outer boundary) is handled by
per-core {0,1} mask inputs multiplied into the halo rows.

All matmuls run as float32r (fp32 data, 1 cycle/row at N>=256,
~1.3e-4 rel err per matmul measured on HW).
"""
import numpy as np

import concourse.bass as bass
import concourse.tile as tile
from concourse import bacc, mybir
from concourse.bass_utils import run_bass_kernel_spmd

T, NB, CIN, C, H, W = 10, 4, 64, 128, 64, 64
BN_EPS = 1e-5
N_CORES = 8
F32 = mybir.dt.float32
F32R = mybir.dt.float32r
MM_DT = F32R  # matmul dtype (flip to F32 for exact-precision fallback)

WP = W + 4        # 68: W padded for 5x5 conv
W2 = W + 2        # 66: W padded for 3x3 conv
XR = 40           # x rows per core (36 y rows need 40 padded x rows)
YR = 36           # y rows per core: [r0-2, r1+2) in global coords
ZR = 34           # zr rows per core: [r0-1, r1+1)
HR = 36           # h_pad rows: [r0-2, r1+2)
OR = 32           # own output rows per core

# conv row-groups (start, nrows) in local y coords [0, 36)
Y_GROUPS = [(0, 8), (8, 8), (16, 8), (24, 8), (32, 4)]
# own rows are yl [2, 34): per-group slices for BN stats (start_in_group, n)
STAT_SLICES = [(2, 6), (0, 8), (0, 8), (0, 8), (0, 2)]
# zr groups in zi' coords [0, 34); interior first, boundary (0 & last) last
ZR_GROUPS = [(8, 8), (16, 8), (24, 6), (0, 8), (30, 4)]
# h_tilde groups in own coords [0, 32)
HT_GROUPS = [(0, 8), (24, 8), (8, 8), (16, 8)]

_CACHE = {}
import os as _os
SCAN_STEPS = int(_os.environ.get("SCAN_STEPS", "9"))


def _build(sim_mode=False):
    nc = bacc.Bacc("TRN2", target_bir_lowering=False, debug=False,
                   num_devices=1 if sim_mode else N_CORES)

    x_d = nc.dram_tensor("x", [T, CIN, XR * WP], F32, kind="ExternalInput")
    wxp_d = nc.dram_tensor("wxp", [128, 2 * 5 * 3 * C], F32, kind="ExternalInput")
    wxs_d = nc.dram_tensor("wxs", [64, 5 * 3 * C], F32, kind="ExternalInput")
    wx4p_d = nc.dram_tensor("wx4p", [128, 2 * 3 * C], F32, kind="ExternalInput")
    wzr_d = nc.dram_tensor("wzr", [128, 9 * 2 * C], F32, kind="ExternalInput")
    whh_d = nc.dram_tensor("whh", [128, 9 * C], F32, kind="ExternalInput")
    # per-channel vectors as [128, 3] (c = partition, couttile = free)
    gamma_d = nc.dram_tensor("gamma3", [128, 3], F32, kind="ExternalInput")
    beta_d = nc.dram_tensor("beta3", [128, 3], F32, kind="ExternalInput")
    bconv_d = nc.dram_tensor("bconv3", [128, 3], F32, kind="ExternalInput")
    ident_d = nc.dram_tensor("ident", [128, 128], F32, kind="ExternalInput")
    mask_d = nc.dram_tensor("mask", [128, 2], F32, kind="ExternalInput")
    zeros_d = nc.dram_tensor("zeros", [128, HR * W2], F32, kind="ExternalInput")
    out_d = nc.dram_tensor("out", [T, C, OR * W], F32, kind="ExternalOutput")

    from contextlib import ExitStack
    with tile.TileContext(nc) as tc:
        with tc.tile_pool(name="singles", bufs=1) as singles, \
             tc.tile_pool(name="dram", bufs=2, space="DRAM") as dram_pool:
            p1ctx = ExitStack()
            xt_pool = p1ctx.enter_context(tc.tile_pool(name="xt", bufs=2))
            stage_pool = p1ctx.enter_context(tc.tile_pool(name="stage", bufs=6))
            ps1 = p1ctx.enter_context(tc.tile_pool(name="ps1", bufs=8, space="PSUM"))

            # ---- load constants / weights ----
            wxp = singles.tile([128, 2, 5, 3 * C], MM_DT)
            nc.sync.dma_start(out=wxp[:], in_=wxp_d.ap().bitcast(MM_DT)
                              .rearrange("p (a b m) -> p a b m", a=2, b=5))
            wxs = singles.tile([64, 5, 3 * C], MM_DT)
            nc.sync.dma_start(out=wxs[:], in_=wxs_d.ap().bitcast(MM_DT)
                              .rearrange("p (b m) -> p b m", b=5))
            wx4p = singles.tile([128, 2, 3 * C], MM_DT)
            nc.sync.dma_start(out=wx4p[:], in_=wx4p_d.ap().bitcast(MM_DT)
                              .rearrange("p (b m) -> p b m", b=2))
            wzr = singles.tile([128, 9, 2 * C], MM_DT)
            nc.sync.dma_start(out=wzr[:], in_=wzr_d.ap().bitcast(MM_DT)
                              .rearrange("p (k m) -> p k m", k=9))
            whh = singles.tile([128, 9, C], MM_DT)
            nc.sync.dma_start(out=whh[:], in_=whh_d.ap().bitcast(MM_DT)
                              .rearrange("p (k m) -> p k m", k=9))
            gamma = singles.tile([128, 3], F32)
            nc.sync.dma_start(out=gamma[:], in_=gamma_d.ap())
            beta = singles.tile([128, 3], F32)
            nc.sync.dma_start(out=beta[:], in_=beta_d.ap())
            bconv = singles.tile([128, 3], F32)
            nc.sync.dma_start(out=bconv[:], in_=bconv_d.ap())
            ident = singles.tile([128, 128], MM_DT)
            nc.sync.dma_start(out=ident[:], in_=ident_d.ap().bitcast(MM_DT))
            mask = singles.tile([128, 2], F32)
            nc.sync.dma_start(out=mask[:], in_=mask_d.ap())

            y_dram = dram_pool.tile([T, 3 * C, YR * W], F32)
            stats = [singles.tile([128, 5 * T, 6], F32, name=f"stats{ct}")
                     for ct in range(3)]

            # ================= Phase 1: x2h conv + BN stats =================
            for t in range(T):
                xt = xt_pool.tile([128, XR * WP], MM_DT, tag="xt")
                nc.sync.dma_start(out=xt[0:64, :],
                                  in_=x_d.ap().bitcast(MM_DT)[t])
                nc.sync.dma_start(out=xt[64:128, 0:(XR - 1) * WP],
                                  in_=x_d.ap().bitcast(MM_DT)[t, :, WP:])
                xt2 = xt_pool.tile([128, XR * WP], MM_DT, tag="xt2")
                nc.sync.dma_start(out=xt2[0:64, :],
                                  in_=x_d.ap().bitcast(MM_DT)[t])
                nc.sync.dma_start(out=xt2[64:128, 0:XR * WP - 1],
                                  in_=x_d.ap().bitcast(MM_DT)[t, :, 1:])
                for ct in range(3):
                    for gi, (yl0, ng) in enumerate(Y_GROUPS):
                        pt = ps1.tile([128, ng, W], F32, tag="p1")
                        nmm = 0
                        for kx in range(5):
                            for p in range(2):
                                src = bass.AP(
                                    tensor=xt.tensor,
                                    offset=xt.offset + (yl0 + 2 * p) * WP + kx,
                                    ap=[[xt.ap[0][0], 128], [WP, ng], [1, W]])
                                nmm += 1
                                nc.tensor.matmul(
                                    pt[:], wxp[:, p, kx, ct * C:(ct + 1) * C],
                                    src, start=(nmm == 1), stop=False)
                        # ky=4 row: col-pairs on xt2 (x | x shifted 1 col)
                        for q in range(2):
                            src = bass.AP(
                                tensor=xt2.tensor,
                                offset=xt2.offset + (yl0 + 4) * WP + 2 * q,
                                ap=[[xt2.ap[0][0], 128], [WP, ng], [1, W]])
                            nc.tensor.matmul(
                                pt[:], wx4p[:, q, ct * C:(ct + 1) * C],
                                src, start=False, stop=False)
                        src = bass.AP(
                            tensor=xt.tensor,
                            offset=xt.offset + (yl0 + 4) * WP + 4,
                            ap=[[xt.ap[0][0], 64], [WP, ng], [1, W]])
                        nc.tensor.matmul(
                            pt[:], wxs[:, 4, ct * C:(ct + 1) * C],
                            src, start=False, stop=True)
                        s0, sn = STAT_SLICES[gi]
                        nc.vector.bn_stats(
                            out=stats[ct][:, t * 5 + gi, :],
                            in_=pt[:, s0:s0 + sn, :]
                            .rearrange("p a b -> p (a b)"))
                        st = stage_pool.tile([128, 8 * W], F32, tag="st")
                        nc.vector.tensor_copy(st[:, 0:ng * W],
                                              pt[:].rearrange("p a b -> p (a b)"))
                        nc.sync.dma_start(
                            out=y_dram[t, ct * C:(ct + 1) * C,
                                       yl0 * W:(yl0 + ng) * W],
                            in_=st[:, 0:ng * W])

            p1ctx.close()
            ysb_pool = tc.alloc_tile_pool(name="ysb", bufs=2)
            ps2 = tc.alloc_tile_pool(name="ps2", bufs=6, space="PSUM")
            work_pool = tc.alloc_tile_pool(name="work", bufs=2)

            # ================= BN: aggregate + AllReduce + affine ===========
            loc = singles.tile([128, 3, 2], F32)
            for ct in range(3):
                nc.vector.bn_aggr(out=loc[:, ct, :], in_=stats[ct][:])
            red = singles.tile([128, 3, 2], F32)
            # col0 = mean, col1 = var + mean^2
            nc.vector.tensor_copy(red[:, :, 0], loc[:, :, 0])
            nc.vector.tensor_mul(red[:, :, 1], loc[:, :, 0], loc[:, :, 0])
            nc.vector.tensor_add(red[:, :, 1], red[:, :, 1], loc[:, :, 1])
            st_in = dram_pool.tile([128, 3, 2], F32)
            st_out = dram_pool.tile([128, 3, 2], F32)
            nc.sync.dma_start(out=st_in[:], in_=red[:])
            if sim_mode:
                nc.sync.dma_start(out=st_out[:], in_=st_in[:])
            else:
                nc.gpsimd.collective_compute(
                    "AllReduce", mybir.AluOpType.add,
                    replica_groups=[list(range(N_CORES))],
                    ins=[st_in.opt()], outs=[st_out.opt()])
            gs = singles.tile([128, 3, 2], F32)
            nc.sync.dma_start(out=gs[:], in_=st_out[:])

            gmean = singles.tile([128, 3], F32)
            nc.scalar.mul(out=gmean[:], in_=gs[:, :, 0], mul=1.0 / N_CORES)
            gvar = singles.tile([128, 3], F32)
            nc.scalar.mul(out=gvar[:], in_=gs[:, :, 1], mul=1.0 / N_CORES)
            mm = singles.tile([128, 3], F32)
            nc.vector.tensor_mul(mm[:], gmean[:], gmean[:])
            nc.vector.tensor_sub(gvar[:], gvar[:], mm[:])
            eps_t = singles.tile([128, 1], F32)
            nc.vector.memset(eps_t[:], BN_EPS)
            sd = singles.tile([128, 3], F32)
            nc.scalar.activation(out=sd[:], in_=gvar[:],
                                 func=mybir.ActivationFunctionType.Sqrt,
                                 bias=eps_t[:])
            rinv = singles.tile([128, 3], F32)
            nc.vector.reciprocal(rinv[:], sd[:])
            a_sc = singles.tile([128, 3], F32)
            nc.vector.tensor_mul(a_sc[:], rinv[:], gamma[:])
            bb = singles.tile([128, 3], F32)
            nc.vector.tensor_mul(bb[:], gmean[:], a_sc[:])
            nc.vector.tensor_sub(bb[:], beta[:], bb[:])
            bstep = singles.tile([128, 3], F32)
            nc.vector.tensor_add(bstep[:], bb[:], bconv[:])
            diag = singles.tile([128, 3, 128], MM_DT)
            for ct in range(3):
                nc.vector.tensor_scalar_mul(diag[:, ct, :], ident[:],
                                            a_sc[:, ct:ct + 1])

            # ================= h0 ==========================================
            h_bufs = [singles.tile([128, HR, W2], MM_DT, name=f"hbuf{i}")
                      for i in range(2)]
            for hb in h_bufs:
                nc.sync.dma_start(out=hb[:].rearrange("p a b -> p (a b)"),
                                  in_=zeros_d.ap().bitcast(MM_DT))
            rh = singles.tile([128, HR, W2], MM_DT)
            nc.sync.dma_start(out=rh[:].rearrange("p a b -> p (a b)"),
                              in_=zeros_d.ap().bitcast(MM_DT))

            ysb = [None, None, None]
            for ct in range(3):
                yt = ysb_pool.tile([128, YR * W], MM_DT, tag=f"ysb{ct}",
                                   name=f"ysb{ct}")
                nc.sync.dma_start(out=yt[:], in_=y_dram[0, ct * C:(ct + 1) * C, :].bitcast(MM_DT))
                ysb[ct] = yt
            sig0 = work_pool.tile([128, YR * W], F32, tag="z", name="sig0")
            nc.scalar.activation(out=sig0[:], in_=ysb[0][:],
                                 func=mybir.ActivationFunctionType.Sigmoid,
                                 bias=bb[:, 0:1], scale=a_sc[:, 0:1])
            tanh0 = work_pool.tile([128, YR * W], F32, tag="r", name="tanh0")
            nc.scalar.activation(out=tanh0[:], in_=ysb[2][:],
                                 func=mybir.ActivationFunctionType.Tanh,
                                 bias=bb[:, 2:3], scale=a_sc[:, 2:3])
            h0 = h_bufs[0]
            nc.vector.tensor_mul(
                h0[:, :, 1:1 + W],
                sig0[:].rearrange("p (a b) -> p a b", a=YR),
                tanh0[:].rearrange("p (a b) -> p a b", a=YR))
            # zero outer junk rows via masks
            nc.vector.tensor_scalar_mul(
                h0[:, 0:2, 1:1 + W], h0[:, 0:2, 1:1 + W], mask[:, 0:1])
            nc.vector.tensor_scalar_mul(
                h0[:, 34:36, 1:1 + W], h0[:, 34:36, 1:1 + W], mask[:, 1:2])
            nc.sync.dma_start(
                out=out_d.ap()[0],
                in_=h0[:, 2:34, 1:1 + W].bitcast(F32))

            # ================= scan steps 1..9 ==============================
            for t in range(1, 1 + SCAN_STEPS):
                h_old = h_bufs[(t - 1) % 2]
                h_new = h_bufs[t % 2]
                if True:
                    for ct in range(3):
                        yt = ysb_pool.tile([128, YR * W], MM_DT,
                                           tag=f"ysb{ct}", name=f"ysb{ct}")
                        nc.sync.dma_start(
                            out=yt[:],
                            in_=y_dram[t, ct * C:(ct + 1) * C, :]
                            .bitcast(MM_DT))
                        ysb[ct] = yt
                z_t = work_pool.tile([128, ZR, W], F32, tag="z")
                r_t = work_pool.tile([128, ZR, W], F32, tag="r")
                # ---- zr conv (z: ct 0, r: ct 1) ----
                for ct in range(2):
                    dst = r_t if ct else z_t
                    for (z0, ng) in ZR_GROUPS:
                        pt = ps2.tile([128, 8, W], F32, tag="p2")
                        nc.tensor.matmul(
                            pt[:, 0:ng, :].rearrange("p a b -> p (a b)"),
                            diag[:, ct, :],
                            ysb[ct][:, (z0 + 1) * W:(z0 + 1 + ng) * W],
                            start=True, stop=False)
                        for k in range(9):
                            ky, kx = divmod(k, 3)
                            src = bass.AP(
                                tensor=h_old.tensor,
                                offset=h_old.offset + (z0 + ky) * W2 + kx,
                                ap=[[h_old.ap[0][0], 128], [W2, ng], [1, W]])
                            nc.tensor.matmul(
                                pt[:, 0:ng, :], wzr[:, k, ct * C:(ct + 1) * C],
                                src, start=False, stop=(k == 8))
                        nc.scalar.activation(
                            out=dst[:, z0:z0 + ng, :], in_=pt[:, 0:ng, :],
                            func=mybir.ActivationFunctionType.Sigmoid,
                            bias=bstep[:, ct:ct + 1])
                        if ct == 1:
                            nc.vector.tensor_mul(
                                rh[:, z0 + 1:z0 + 1 + ng, 1:1 + W],
                                r_t[:, z0:z0 + ng, :],
                                h_old[:, z0 + 1:z0 + 1 + ng, 1:1 + W])
                # ---- h_tilde conv + h update ----
                cin = dram_pool.tile([128, 4, W], F32, tag="cin")
                for (o0, ng) in HT_GROUPS:
                    pt = ps2.tile([128, 8, W], F32, tag="p2")
                    nc.tensor.matmul(
                        pt[:].rearrange("p a b -> p (a b)"),
                        diag[:, 2, :],
                        ysb[2][:, (o0 + 2) * W:(o0 + 2 + ng) * W],
                        start=True, stop=False)
                    for k in range(9):
                        ky, kx = divmod(k, 3)
                        src = bass.AP(
                            tensor=rh.tensor,
                            offset=rh.offset + (o0 + ky + 1) * W2 + kx,
                            ap=[[rh.ap[0][0], 128], [W2, ng], [1, W]])
                        nc.tensor.matmul(
                            pt[:], whh[:, k, :], src,
                            start=False, stop=(k == 8))
                    ht = work_pool.tile([128, 8, W], F32, tag="ht")
                    nc.scalar.activation(
                        out=ht[:], in_=pt[:],
                        func=mybir.ActivationFunctionType.Tanh,
                        bias=bstep[:, 2:3])
                    # h_new[2+o0 : 2+o0+ng) = h_old + z*(ht - h_old)
                    hp0 = 2 + o0
                    d_t = work_pool.tile([128, 8, W], F32, tag="d")
                    nc.vector.tensor_sub(
                        d_t[:], ht[:], h_old[:, hp0:hp0 + ng, 1:1 + W])
                    nc.vector.tensor_mul(
                        d_t[:], d_t[:], z_t[:, o0 + 1:o0 + 1 + ng, :])
                    nc.vector.tensor_add(
                        h_new[:, hp0:hp0 + ng, 1:1 + W],
                        h_old[:, hp0:hp0 + ng, 1:1 + W], d_t[:])
                    if o0 == 0 and t < T:
                        nc.sync.dma_start(
                            out=cin[:, 0:2, :],
                            in_=h_new[:, 2:4, 1:1 + W].bitcast(F32))
                    if o0 == 24 and t < T:
                        nc.sync.dma_start(
                            out=cin[:, 2:4, :],
                            in_=h_new[:, 32:34, 1:1 + W].bitcast(F32))
                # halo exchange: send own top2 (hp 2:4) + bottom2 (hp 32:34)
                if t < T:
                    cout = dram_pool.tile([2, 128, 4, W], F32, tag="cout")
                    if sim_mode:
                        nc.sync.dma_start(out=cout[0], in_=cin[:])
                        nc.sync.dma_start(out=cout[1], in_=cin[:])
                    else:
                        nc.gpsimd.collective_compute(
                            "AllGather", mybir.AluOpType.bypass,
                            replica_groups=[[0, 1], [2, 3], [4, 5], [6, 7]],
                            ins=[cin.opt()], outs=[cout.opt()])
                    halo = work_pool.tile([128, 4, W], F32, tag="halo")
                    nc.sync.dma_start(out=halo[:, 0:2, :],
                                      in_=cout[0, :, 2:4, :])
                    nc.sync.dma_start(out=halo[:, 2:4, :],
                                      in_=cout[1, :, 0:2, :])
                    nc.vector.tensor_scalar_mul(
                        h_new[:, 0:2, 1:1 + W], halo[:, 0:2, :], mask[:, 0:1])
                    nc.vector.tensor_scalar_mul(
                        h_new[:, 34:36, 1:1 + W], halo[:, 2:4, :],
                        mask[:, 1:2])
                nc.sync.dma_start(
                    out=out_d.ap()[t],
                    in_=h_new[:, 2:34, 1:1 + W].bitcast(F32))
            work_pool.release()
            ps2.release()
            ysb_pool.release()
    nc.compile()
    return nc


def _get_nc():
    if "nc" not in _CACHE:
        _CACHE["nc"] = _build()
    return _CACHE["nc"]


def _make_in_maps(inputs):
    x = np.asarray(inputs["x"], dtype=np.float32)
    w_x2h = np.asarray(inputs["w_x2h"], dtype=np.float32)
    gamma = np.asarray(inputs["gamma"], dtype=np.float32)
    beta = np.asarray(inputs["beta"], dtype=np.float32)
    w_h2zr = np.asarray(inputs["w_h2zr"], dtype=np.float32)
    b_h2zr = np.asarray(inputs["b_h2zr"], dtype=np.float32)
    w_h2h = np.asarray(inputs["w_h2h"], dtype=np.float32)
    b_h2h = np.asarray(inputs["b_h2h"], dtype=np.float32)

    xp = np.pad(x, ((0, 0), (0, 0), (0, 0), (4, 4), (2, 2)))
    # wxp[k, p, kx, m]: k<64: w[m, k, 2p, kx]; k>=64: w[m, k-64, 2p+1, kx]
    wxp = np.zeros((128, 2, 5, 3 * C), np.float32)
    for p in range(2):
        wxp[0:64, p] = w_x2h[:, :, 2 * p, :].transpose(1, 2, 0)
        wxp[64:128, p] = w_x2h[:, :, 2 * p + 1, :].transpose(1, 2, 0)
    wxs = np.ascontiguousarray(
        w_x2h[:, :, 4, :].transpose(1, 2, 0))          # [64, 5, 384]
    wx4p = np.zeros((128, 2, 3 * C), np.float32)
    for q in range(2):
        wx4p[0:64, q] = w_x2h[:, :, 4, 2 * q].T
        wx4p[64:128, q] = w_x2h[:, :, 4, 2 * q + 1].T
    wzr = np.ascontiguousarray(
        w_h2zr.reshape(2 * C, C, 9).transpose(1, 2, 0))  # [128, 9, 256]
    whh = np.ascontiguousarray(
        w_h2h.reshape(C, C, 9).transpose(1, 2, 0))       # [128, 9, 128]
    gamma3 = np.ascontiguousarray(gamma.reshape(3, 128).T)
    beta3 = np.ascontiguousarray(beta.reshape(3, 128).T)
    bconv3 = np.stack([b_h2zr[0:128], b_h2zr[128:256], b_h2h], axis=1)
    ident = np.eye(128, dtype=np.float32)

    in_maps = []
    for j in range(N_CORES):
        n, half = j // 2, j % 2
        r0 = half * OR
        x_sh = np.ascontiguousarray(
            xp[:, n, :, r0:r0 + XR, :].reshape(T, CIN, XR * WP))
        msk = np.zeros((128, 2), np.float32)
        msk[:, 0] = 1.0 if half == 1 else 0.0   # below-neighbor exists
        msk[:, 1] = 1.0 if half == 0 else 0.0   # above-neighbor exists
        in_maps.append({
            "x": x_sh,
            "wxp": wxp.reshape(128, -1), "wxs": wxs.reshape(64, -1),
            "wx4p": wx4p.reshape(128, -1),
            "wzr": wzr.reshape(128, -1), "whh": whh.reshape(128, -1),
            "gamma3": gamma3, "beta3": beta3, "bconv3": bconv3,
            "ident": ident, "mask": msk,
            "zeros": np.zeros((128, HR * W2), np.float32),
        })
    return in_maps


def _gather_out(results):
    out = np.empty((T, NB, C, H, W), np.float32)
    for j in range(N_CORES):
        n, half = j // 2, j % 2
        r0 = half * OR
        out[:, n, :, r0:r0 + OR, :] = \
            results[j]["out"].reshape(T, C, OR, W)
    return out


def kernel(x, w_x2h, b_x2h, gamma, beta, w_h2zr, b_h2zr, w_h2h, b_h2h):
    nc = _get_nc()
    in_maps = _make_in_maps(dict(
        x=x, w_x2h=w_x2h, b_x2h=b_x2h, gamma=gamma, beta=beta,
        w_h2zr=w_h2zr, b_h2zr=b_h2zr, w_h2h=w_h2h, b_h2h=b_h2h))
    res = run_bass_kernel_spmd(nc, in_maps, list(range(N_CORES)))
    return _gather_out(res.results)



# revision 5
# speedup vs baseline: 74.5462x; 74.5462x over previous
"""ConvGRU Trainium2 kernel (8 NeuronCores, SPMD) — v2.

Problem: T=10, N=4, CIN=64, C=128, H=W=64.
  y = BN(conv5x5(x))  over T*N batch  -> GRU scan over T with conv3x3 gates.

Sharding: 8 cores = N(4) x H-halves(2). Core j: n=j//2, half=j%2,
rows [r0,r1) = [0,32) or [32,64). Phase 1 (x2h conv + BN stats) is fully
local per core (input halos come free from host-side slicing); BN stats
need one 8-core AllReduce; the scan exchanges a 2-row h halo between
H-half partners each step (pairwise AllGather).

v2 changes vs v1 (1001us HW):
 - all matmul operands bf16 (fp32r LDWEIGHTS serializes ~10ns/mm more,
   and bf16 halves DMA); h state stays fp32 with a bf16 shadow for the
   conv inputs, so the recurrence does not accumulate rounding.
 - zr groups interleave ct0/ct1 with all interior groups first so the
   halo AllGather latency (~13us) is covered by ~16us of matmuls.
 - the final step's halo exchange (never read) is dropped.
 - phase-1 weights load before the first x tile; scan weights load
   behind phase 1's compute.

The program is identical on all cores (SPMD). Half-dependent behavior
(which halo rows are real vs zero at the outer boundary) is handled by
per-core {0,1} mask inputs multiplied into the halo rows.
"""
import numpy as np

import concourse.bass as bass
import concourse.tile as tile
from concourse import bacc, mybir
from concourse.bass_utils import run_bass_kernel_spmd

T, NB, CIN, C, H, W = 10, 4, 64, 128, 64, 64
BN_EPS = 1e-5
N_CORES = 8
F32 = mybir.dt.float32
BF16 = mybir.dt.bfloat16

WP = W + 4        # 68: W padded for 5x5 conv
W2 = W + 2        # 66: W padded for 3x3 conv
XR = 40           # x rows per core (36 y rows need 40 padded x rows)
YR = 36           # y rows per core: [r0-2, r1+2) in global coords
ZR = 34           # zr rows per core: [r0-1, r1+1)
HR = 36           # h_pad rows: [r0-2, r1+2)
OR = 32           # own output rows per core

# conv row-groups (start, nrows) in local y coords [0, 36)
Y_GROUPS = [(0, 8), (8, 8), (16, 8), (24, 8), (32, 4)]
# own rows are yl [2, 34): per-group slices for BN stats (start_in_group, n)
STAT_SLICES = [(2, 6), (0, 8), (0, 8), (0, 8), (0, 2)]
# zr groups in zi' coords [0, 34); interior first, boundary (0 & last) last
ZR_GROUPS = [(8, 8), (16, 8), (24, 6), (0, 8), (30, 4)]
# h_tilde groups in own coords [0, 32): halo-source groups (0, 24) first
HT_GROUPS = [(0, 8), (24, 8), (8, 8), (16, 8)]

_CACHE = {}
import os as _os
SCAN_STEPS = int(_os.environ.get("SCAN_STEPS", "9"))


def _build(sim_mode=False):
    nc = bacc.Bacc("TRN2", target_bir_lowering=False, debug=False,
                   num_devices=1 if sim_mode else N_CORES)

    x_d = nc.dram_tensor("x", [T, CIN, XR * WP], BF16, kind="ExternalInput")
    wxp_d = nc.dram_tensor("wxp", [128, 2 * 5 * 3 * C], BF16,
                           kind="ExternalInput")
    wxs_d = nc.dram_tensor("wxs", [64, 3 * C], BF16, kind="ExternalInput")
    wx4p_d = nc.dram_tensor("wx4p", [128, 2 * 3 * C], BF16,
                            kind="ExternalInput")
    wzr_d = nc.dram_tensor("wzr", [128, 9 * 2 * C], BF16,
                           kind="ExternalInput")
    whh_d = nc.dram_tensor("whh", [128, 9 * C], BF16, kind="ExternalInput")
    # per-channel vectors as [128, 3] (c = partition, couttile = free)
    gamma_d = nc.dram_tensor("gamma3", [128, 3], F32, kind="ExternalInput")
    beta_d = nc.dram_tensor("beta3", [128, 3], F32, kind="ExternalInput")
    bconv_d = nc.dram_tensor("bconv3", [128, 3], F32, kind="ExternalInput")
    ident_d = nc.dram_tensor("ident", [128, 128], BF16, kind="ExternalInput")
    mask_d = nc.dram_tensor("mask", [128, 2], F32, kind="ExternalInput")
    out_d = nc.dram_tensor("out", [T, C, OR * W], F32, kind="ExternalOutput")

    from contextlib import ExitStack
    with tile.TileContext(nc) as tc:
        with tc.tile_pool(name="singles", bufs=1) as singles, \
             tc.tile_pool(name="dram", bufs=2, space="DRAM") as dram_pool:
            p1ctx = ExitStack()
            xt_pool = p1ctx.enter_context(tc.tile_pool(name="xt", bufs=2))
            stage_pool = p1ctx.enter_context(tc.tile_pool(name="stage",
                                                          bufs=6))
            ps1 = p1ctx.enter_context(tc.tile_pool(name="ps1", bufs=8,
                                                   space="PSUM"))

            # ---- phase-1 weights first (the first matmul needs them) ----
            wxp = singles.tile([128, 2, 5, 3 * C], BF16)
            nc.sync.dma_start(out=wxp[:], in_=wxp_d.ap()
                              .rearrange("p (a b m) -> p a b m", a=2, b=5))
            wxs = singles.tile([64, 3 * C], BF16)
            nc.sync.dma_start(out=wxs[:], in_=wxs_d.ap())
            wx4p = singles.tile([128, 2, 3 * C], BF16)
            nc.sync.dma_start(out=wx4p[:], in_=wx4p_d.ap()
                              .rearrange("p (b m) -> p b m", b=2))

            y_dram = dram_pool.tile([T, 3 * C, YR * W], BF16)
            stats = [singles.tile([128, 5 * T, 6], F32, name=f"stats{ct}")
                     for ct in range(3)]

            # scan weights + small vectors: emitted after t=0's x tiles so
            # they ride behind phase-1 compute on the DMA queue.
            deferred = []

            # ================= Phase 1: x2h conv + BN stats =================
            for t in range(T):
                xt = xt_pool.tile([128, XR * WP], BF16, tag="xt")
                nc.sync.dma_start(out=xt[0:64, :], in_=x_d.ap()[t])
                nc.sync.dma_start(out=xt[64:128, 0:(XR - 1) * WP],
                                  in_=x_d.ap()[t, :, WP:])
                xt2 = xt_pool.tile([128, XR * WP], BF16, tag="xt2")
                nc.sync.dma_start(out=xt2[0:64, :], in_=x_d.ap()[t])
                nc.sync.dma_start(out=xt2[64:128, 0:XR * WP - 1],
                                  in_=x_d.ap()[t, :, 1:])
                if t == 0:
                    # big scan weights: behind the t=0 x tiles
                    wzr = singles.tile([128, 9, 2 * C], BF16)
                    nc.sync.dma_start(out=wzr[:], in_=wzr_d.ap()
                                      .rearrange("p (k m) -> p k m", k=9))
                    whh = singles.tile([128, 9, C], BF16)
                    nc.sync.dma_start(out=whh[:], in_=whh_d.ap()
                                      .rearrange("p (k m) -> p k m", k=9))
                    gamma = singles.tile([128, 3], F32)
                    nc.sync.dma_start(out=gamma[:], in_=gamma_d.ap())
                    beta = singles.tile([128, 3], F32)
                    nc.sync.dma_start(out=beta[:], in_=beta_d.ap())
                    bconv = singles.tile([128, 3], F32)
                    nc.sync.dma_start(out=bconv[:], in_=bconv_d.ap())
                    ident = singles.tile([128, 128], BF16)
                    nc.sync.dma_start(out=ident[:], in_=ident_d.ap())
                    mask = singles.tile([128, 2], F32)
                    nc.sync.dma_start(out=mask[:], in_=mask_d.ap())
                for ct in range(3):
                    for gi, (yl0, ng) in enumerate(Y_GROUPS):
                        pt = ps1.tile([128, ng, W], F32, tag="p1")
                        nmm = 0
                        for kx in range(5):
                            for p in range(2):
                                src = bass.AP(
                                    tensor=xt.tensor,
                                    offset=xt.offset + (yl0 + 2 * p) * WP + kx,
                                    ap=[[xt.ap[0][0], 128], [WP, ng], [1, W]])
                                nmm += 1
                                nc.tensor.matmul(
                                    pt[:], wxp[:, p, kx, ct * C:(ct + 1) * C],
                                    src, start=(nmm == 1), stop=False)
                        # ky=4 row: col-pairs on xt2 (x | x shifted 1 col)
                        for q in range(2):
                            src = bass.AP(
                                tensor=xt2.tensor,
                                offset=xt2.offset + (yl0 + 4) * WP + 2 * q,
                                ap=[[xt2.ap[0][0], 128], [WP, ng], [1, W]])
                            nc.tensor.matmul(
                                pt[:], wx4p[:, q, ct * C:(ct + 1) * C],
                                src, start=False, stop=False)
                        src = bass.AP(
                            tensor=xt.tensor,
                            offset=xt.offset + (yl0 + 4) * WP + 4,
                            ap=[[xt.ap[0][0], 64], [WP, ng], [1, W]])
                        nc.tensor.matmul(
                            pt[:], wxs[:, ct * C:(ct + 1) * C],
                            src, start=False, stop=True)
                        s0, sn = STAT_SLICES[gi]
                        nc.vector.bn_stats(
                            out=stats[ct][:, t * 5 + gi, :],
                            in_=pt[:, s0:s0 + sn, :]
                            .rearrange("p a b -> p (a b)"))
                        st = stage_pool.tile([128, 8 * W], BF16, tag="st")
                        nc.vector.tensor_copy(st[:, 0:ng * W],
                                              pt[:].rearrange(
                                                  "p a b -> p (a b)"))
                        nc.sync.dma_start(
                            out=y_dram[t, ct * C:(ct + 1) * C,
                                       yl0 * W:(yl0 + ng) * W],
                            in_=st[:, 0:ng * W])

            p1ctx.close()
            ysb_pool = tc.alloc_tile_pool(name="ysb", bufs=3)
            ps2 = tc.alloc_tile_pool(name="ps2", bufs=6, space="PSUM")
            work_pool = tc.alloc_tile_pool(name="work", bufs=2)

            # h state: fp32, updated in place; hbf: bf16 shadow (conv input)
            h_f = singles.tile([128, HR, W2], F32)
            nc.vector.memset(h_f[:], 0.0)
            hbf = singles.tile([128, HR, W2], BF16)
            nc.vector.memset(hbf[:], 0.0)
            rh = singles.tile([128, HR, W2], BF16)
            nc.vector.memset(rh[:], 0.0)

            # prefetch y[0] for h0 (not BN-dependent)
            ysb = [None, None, None]
            for ct in range(3):
                yt = ysb_pool.tile([128, YR * W], BF16, tag=f"ysb{ct}",
                                   name=f"ysb{ct}")
                nc.sync.dma_start(out=yt[:],
                                  in_=y_dram[0, ct * C:(ct + 1) * C, :])
                ysb[ct] = yt

            # ================= BN: aggregate + AllReduce + affine ===========
            loc = singles.tile([128, 3, 2], F32)
            for ct in range(3):
                nc.vector.bn_aggr(out=loc[:, ct, :], in_=stats[ct][:])
            red = singles.tile([128, 3, 2], F32)
            # col0 = mean, col1 = var + mean^2
            nc.vector.tensor_copy(red[:, :, 0], loc[:, :, 0])
            nc.vector.tensor_mul(red[:, :, 1], loc[:, :, 0], loc[:, :, 0])
            nc.vector.tensor_add(red[:, :, 1], red[:, :, 1], loc[:, :, 1])
            st_in = dram_pool.tile([128, 3, 2], F32)
            st_out = dram_pool.tile([128, 3, 2], F32)
            nc.sync.dma_start(out=st_in[:], in_=red[:])
            if sim_mode:
                nc.sync.dma_start(out=st_out[:], in_=st_in[:])
            else:
                nc.gpsimd.collective_compute(
                    "AllReduce", mybir.AluOpType.add,
                    replica_groups=[list(range(N_CORES))],
                    ins=[st_in.opt()], outs=[st_out.opt()])
            gs = singles.tile([128, 3, 2], F32)
            nc.sync.dma_start(out=gs[:], in_=st_out[:])

            gmean = singles.tile([128, 3], F32)
            nc.scalar.mul(out=gmean[:], in_=gs[:, :, 0], mul=1.0 / N_CORES)
            gvar = singles.tile([128, 3], F32)
            nc.scalar.mul(out=gvar[:], in_=gs[:, :, 1], mul=1.0 / N_CORES)
            mm = singles.tile([128, 3], F32)
            nc.vector.tensor_mul(mm[:], gmean[:], gmean[:])
            nc.vector.tensor_sub(gvar[:], gvar[:], mm[:])
            eps_t = singles.tile([128, 1], F32)
            nc.vector.memset(eps_t[:], BN_EPS)
            sd = singles.tile([128, 3], F32)
            nc.scalar.activation(out=sd[:], in_=gvar[:],
                                 func=mybir.ActivationFunctionType.Sqrt,
                                 bias=eps_t[:])
            rinv = singles.tile([128, 3], F32)
            nc.vector.reciprocal(rinv[:], sd[:])
            a_sc = singles.tile([128, 3], F32)
            nc.vector.tensor_mul(a_sc[:], rinv[:], gamma[:])
            bb = singles.tile([128, 3], F32)
            nc.vector.tensor_mul(bb[:], gmean[:], a_sc[:])
            nc.vector.tensor_sub(bb[:], beta[:], bb[:])
            bstep = singles.tile([128, 3], F32)
            nc.vector.tensor_add(bstep[:], bb[:], bconv[:])
            diag = singles.tile([128, 3, 128], BF16)
            for ct in range(3):
                nc.vector.tensor_scalar_mul(diag[:, ct, :], ident[:],
                                            a_sc[:, ct:ct + 1])

            # ================= h0 ==========================================
            sig0 = work_pool.tile([128, YR * W], F32, tag="z", name="sig0")
            nc.scalar.activation(out=sig0[:], in_=ysb[0][:],
                                 func=mybir.ActivationFunctionType.Sigmoid,
                                 bias=bb[:, 0:1], scale=a_sc[:, 0:1])
            tanh0 = work_pool.tile([128, YR * W], F32, tag="r", name="tanh0")
            nc.scalar.activation(out=tanh0[:], in_=ysb[2][:],
                                 func=mybir.ActivationFunctionType.Tanh,
                                 bias=bb[:, 2:3], scale=a_sc[:, 2:3])
            nc.vector.tensor_mul(
                h_f[:, :, 1:1 + W],
                sig0[:].rearrange("p (a b) -> p a b", a=YR),
                tanh0[:].rearrange("p (a b) -> p a b", a=YR))
            # zero outer junk rows via masks
            nc.vector.tensor_scalar_mul(
                h_f[:, 0:2, 1:1 + W], h_f[:, 0:2, 1:1 + W], mask[:, 0:1])
            nc.vector.tensor_scalar_mul(
                h_f[:, 34:36, 1:1 + W], h_f[:, 34:36, 1:1 + W], mask[:, 1:2])
            nc.vector.tensor_copy(hbf[:, :, 1:1 + W], h_f[:, :, 1:1 + W])
            nc.sync.dma_start(out=out_d.ap()[0], in_=h_f[:, 2:34, 1:1 + W])
            # prefetch y[1] for step 1
            for ct in range(3):
                yt = ysb_pool.tile([128, YR * W], BF16,
                                   tag=f"ysb{ct}", name=f"ysb{ct}")
                nc.sync.dma_start(out=yt[:],
                                  in_=y_dram[1, ct * C:(ct + 1) * C, :])
                ysb[ct] = yt

            # ================= scan steps 1..9 ==============================
            for t in range(1, 1 + SCAN_STEPS):
                last = (t == SCAN_STEPS)
                z_t = work_pool.tile([128, ZR, W], F32, tag="z")
                r_t = work_pool.tile([128, ZR, W], F32, tag="r")
                # ---- zr conv: interleave ct0/ct1 per group, interior
                #      groups of both cts before any boundary group ----
                for (z0, ng) in ZR_GROUPS:
                    for ct in range(2):
                        dst = r_t if ct else z_t
                        pt = ps2.tile([128, 8, W], F32, tag="p2")
                        nc.tensor.matmul(
                            pt[:, 0:ng, :].rearrange("p a b -> p (a b)"),
                            diag[:, ct, :],
                            ysb[ct][:, (z0 + 1) * W:(z0 + 1 + ng) * W],
                            start=True, stop=False)
                        for k in range(9):
                            ky, kx = divmod(k, 3)
                            src = bass.AP(
                                tensor=hbf.tensor,
                                offset=hbf.offset + (z0 + ky) * W2 + kx,
                                ap=[[hbf.ap[0][0], 128], [W2, ng], [1, W]])
                            nc.tensor.matmul(
                                pt[:, 0:ng, :],
                                wzr[:, k, ct * C:(ct + 1) * C],
                                src, start=False, stop=(k == 8))
                        nc.scalar.activation(
                            out=dst[:, z0:z0 + ng, :], in_=pt[:, 0:ng, :],
                            func=mybir.ActivationFunctionType.Sigmoid,
                            bias=bstep[:, ct:ct + 1])
                        if ct == 1:
                            nc.vector.tensor_mul(
                                rh[:, z0 + 1:z0 + 1 + ng, 1:1 + W],
                                r_t[:, z0:z0 + ng, :],
                                h_f[:, z0 + 1:z0 + 1 + ng, 1:1 + W])
                # ---- h_tilde conv + in-place h update ----
                cin = dram_pool.tile([128, 4, W], F32, tag="cin")
                for (o0, ng) in HT_GROUPS:
                    pt = ps2.tile([128, 8, W], F32, tag="p2")
                    nc.tensor.matmul(
                        pt[:].rearrange("p a b -> p (a b)"),
                        diag[:, 2, :],
                        ysb[2][:, (o0 + 2) * W:(o0 + 2 + ng) * W],
                        start=True, stop=False)
                    for k in range(9):
                        ky, kx = divmod(k, 3)
                        src = bass.AP(
                            tensor=rh.tensor,
                            offset=rh.offset + (o0 + ky + 1) * W2 + kx,
                            ap=[[rh.ap[0][0], 128], [W2, ng], [1, W]])
                        nc.tensor.matmul(
                            pt[:], whh[:, k, :], src,
                            start=False, stop=(k == 8))
                    ht = work_pool.tile([128, 8, W], F32, tag="ht")
                    nc.scalar.activation(
                        out=ht[:], in_=pt[:],
                        func=mybir.ActivationFunctionType.Tanh,
                        bias=bstep[:, 2:3])
                    # h[2+o0 : 2+o0+ng) += z * (ht - h)   (in place)
                    hp0 = 2 + o0
                    d_t = work_pool.tile([128, 8, W], F32, tag="d")
                    nc.vector.tensor_sub(
                        d_t[:], ht[:], h_f[:, hp0:hp0 + ng, 1:1 + W])
                    nc.vector.tensor_mul(
                        d_t[:], d_t[:], z_t[:, o0 + 1:o0 + 1 + ng, :])
                    nc.vector.tensor_add(
                        h_f[:, hp0:hp0 + ng, 1:1 + W],
                        h_f[:, hp0:hp0 + ng, 1:1 + W], d_t[:])
                    nc.vector.tensor_copy(
                        hbf[:, hp0:hp0 + ng, 1:1 + W],
                        h_f[:, hp0:hp0 + ng, 1:1 + W])
                    if o0 == 0 and not last:
                        nc.sync.dma_start(out=cin[:, 0:2, :],
                                          in_=h_f[:, 2:4, 1:1 + W])
                    if o0 == 24 and not last:
                        nc.sync.dma_start(out=cin[:, 2:4, :],
                                          in_=h_f[:, 32:34, 1:1 + W])
                # out + next-step y prefetch BEFORE the halo-in DMAs, so the
                # halo's collective wait can't head-of-line-block them.
                nc.sync.dma_start(out=out_d.ap()[t],
                                  in_=h_f[:, 2:34, 1:1 + W])
                if not last:
                    for ct in range(3):
                        yt = ysb_pool.tile([128, YR * W], BF16,
                                           tag=f"ysb{ct}", name=f"ysb{ct}")
                        nc.sync.dma_start(
                            out=yt[:],
                            in_=y_dram[t + 1, ct * C:(ct + 1) * C, :])
                        ysb[ct] = yt
                # halo exchange: send own top2 (hp 2:4) + bottom2 (hp 32:34)
                if not last:
                    cout = dram_pool.tile([2, 128, 4, W], F32, tag="cout")
                    if sim_mode:
                        nc.sync.dma_start(out=cout[0], in_=cin[:])
                        nc.sync.dma_start(out=cout[1], in_=cin[:])
                    else:
                        nc.gpsimd.collective_compute(
                            "AllGather", mybir.AluOpType.bypass,
                            replica_groups=[[0, 1], [2, 3], [4, 5], [6, 7]],
                            ins=[cin.opt()], outs=[cout.opt()])
                    halo = work_pool.tile([128, 4, W], F32, tag="halo")
                    nc.sync.dma_start(out=halo[:, 0:2, :],
                                      in_=cout[0, :, 2:4, :])
                    nc.sync.dma_start(out=halo[:, 2:4, :],
                                      in_=cout[1, :, 0:2, :])
                    nc.vector.tensor_scalar_mul(
                        h_f[:, 0:2, 1:1 + W], halo[:, 0:2, :], mask[:, 0:1])
                    nc.vector.tensor_scalar_mul(
                        h_f[:, 34:36, 1:1 + W], halo[:, 2:4, :],
                        mask[:, 1:2])
                    nc.vector.tensor_copy(hbf[:, 0:2, 1:1 + W],
                                          h_f[:, 0:2, 1:1 + W])
                    nc.vector.tensor_copy(hbf[:, 34:36, 1:1 + W],
                                          h_f[:, 34:36, 1:1 + W])
            work_pool.release()
            ps2.release()
            ysb_pool.release()
    nc.compile()
    return nc


def _get_nc():
    if "nc" not in _CACHE:
        _CACHE["nc"] = _build()
    return _CACHE["nc"]


def _bf16(a):
    import ml_dtypes
    return np.ascontiguousarray(a.astype(ml_dtypes.bfloat16))


def _make_in_maps(inputs):
    x = np.asarray(inputs["x"], dtype=np.float32)
    w_x2h = np.asarray(inputs["w_x2h"], dtype=np.float32)
    gamma = np.asarray(inputs["gamma"], dtype=np.float32)
    beta = np.asarray(inputs["beta"], dtype=np.float32)
    w_h2zr = np.asarray(inputs["w_h2zr"], dtype=np.float32)
    b_h2zr = np.asarray(inputs["b_h2zr"], dtype=np.float32)
    w_h2h = np.asarray(inputs["w_h2h"], dtype=np.float32)
    b_h2h = np.asarray(inputs["b_h2h"], dtype=np.float32)

    xp = np.pad(x, ((0, 0), (0, 0), (0, 0), (4, 4), (2, 2)))
    # wxp[k, p, kx, m]: k<64: w[m, k, 2p, kx]; k>=64: w[m, k-64, 2p+1, kx]
    wxp = np.zeros((128, 2, 5, 3 * C), np.float32)
    for p in range(2):
        wxp[0:64, p] = w_x2h[:, :, 2 * p, :].transpose(1, 2, 0)
        wxp[64:128, p] = w_x2h[:, :, 2 * p + 1, :].transpose(1, 2, 0)
    wxs = np.ascontiguousarray(w_x2h[:, :, 4, 4].T)       # [64, 384]
    wx4p = np.zeros((128, 2, 3 * C), np.float32)
    for q in range(2):
        wx4p[0:64, q] = w_x2h[:, :, 4, 2 * q].T
        wx4p[64:128, q] = w_x2h[:, :, 4, 2 * q + 1].T
    wzr = np.ascontiguousarray(
        w_h2zr.reshape(2 * C, C, 9).transpose(1, 2, 0))  # [128, 9, 256]
    whh = np.ascontiguousarray(
        w_h2h.reshape(C, C, 9).transpose(1, 2, 0))       # [128, 9, 128]
    gamma3 = np.ascontiguousarray(gamma.reshape(3, 128).T)
    beta3 = np.ascontiguousarray(beta.reshape(3, 128).T)
    bconv3 = np.stack([b_h2zr[0:128], b_h2zr[128:256], b_h2h], axis=1)
    ident = np.eye(128, dtype=np.float32)

    wxp_b = _bf16(wxp.reshape(128, -1))
    wxs_b = _bf16(wxs)
    wx4p_b = _bf16(wx4p.reshape(128, -1))
    wzr_b = _bf16(wzr.reshape(128, -1))
    whh_b = _bf16(whh.reshape(128, -1))
    ident_b = _bf16(ident)

    in_maps = []
    for j in range(N_CORES):
        n, half = j // 2, j % 2
        r0 = half * OR
        x_sh = _bf16(xp[:, n, :, r0:r0 + XR, :].reshape(T, CIN, XR * WP))
        msk = np.zeros((128, 2), np.float32)
        msk[:, 0] = 1.0 if half == 1 else 0.0   # below-neighbor exists
        msk[:, 1] = 1.0 if half == 0 else 0.0   # above-neighbor exists
        in_maps.append({
            "x": x_sh,
            "wxp": wxp_b, "wxs": wxs_b, "wx4p": wx4p_b,
            "wzr": wzr_b, "whh": whh_b,
            "gamma3": gamma3, "beta3": beta3, "bconv3": bconv3,
            "ident": ident_b, "mask": msk,
        })
    return in_maps


def _gather_out(results):
    out = np.empty((T, NB, C, H, W), np.float32)
    for j in range(N_CORES):
        n, half = j // 2, j % 2
        r0 = half * OR
        out[:, n, :, r0:r0 + OR, :] = \
            results[j]["out"].reshape(T, C, OR, W)
    return out


def kernel(x, w_x2h, b_x2h, gamma, beta, w_h2zr, b_h2zr, w_h2h, b_h2h):
    nc = _get_nc()
    in_maps = _make_in_maps(dict(
        x=x, w_x2h=w_x2h, b_x2h=b_x2h, gamma=gamma, beta=beta,
        w_h2zr=w_h2zr, b_h2zr=b_h2zr, w_h2h=w_h2h, b_h2h=b_h2h))
    res = run_bass_kernel_spmd(nc, in_maps, list(range(N_CORES)))
    return _gather_out(res.results)
